# revision 18
# baseline (speedup 1.0000x reference)
"""Trainium2 Bass kernel: per-image segment-mean repaint (DeepgazeSpade).

Reference computation per image b:
  seg_ds        = segmap[::8, ::8]                  (nearest downsample: 384/48 = 512/64 = 8)
  sums[s, c]    = sum_{p : seg_ds[p] == s} feats[c, p]
  counts[s]     = |{p : seg_ds[p] == s}|
  avg[s, c]     = sums / max(counts, 1)             (0 for empty segments)
  out[c, y, x]  = avg[segmap[y, x], c]

Sharding: 8 cores = 4 images x 2 row-halves (pure data parallel, no
collectives). Each core recomputes the (cheap) per-image segment stats from
the full downsampled grid and paints its own half of the image.

Per-core device algorithm:
  stats: PE-transpose feats [c, pix] -> [pix, c] tiles; build the one-hot
         [pix, s] with tensor_scalar(is_equal) against an iota-row constant;
         24 accumulating matmuls produce [s, 256 sums + counts] in PSUM;
         avg = sums * reciprocal(max(counts, 1)).
  paint: for each 512-pixel tile: a K=1 matmul broadcasts the seg ids to all
         128 partitions (PSUM); tensor_scalar(is_equal) against the
         partition-index iota gives the one-hot [s=128, 512] in SBUF; two
         float32r matmuls with avg as the stationary operand gather all 256
         channels (exact fp32 selection); copy PSUM->SBUF; DMA out.
"""

import numpy as np

B, C = 4, 256
HF, WF = 48, 64
HIMG, WIMG = 384, 512
S = 128
NPIX_DS = HF * WF              # 3072 downsampled pixels
NCHUNK_DS = NPIX_DS // 128     # 24
HALF_ROWS = HIMG // 2          # 192
NPIX_HALF = HALF_ROWS * WIMG   # 98304 pixels per core
SEG_P = 96                     # seg_half SBUF partitions
SEG_F = NPIX_HALF // SEG_P     # 1024
TILE = 512                     # paint tile (one PSUM bank of fp32)
NTILES = NPIX_HALF // TILE     # 192

_CACHE = {}
LAST_RESULTS = None
TRACE = False


WIN = 8192                     # seg-id window on partition 0 (bf16, 16 KiB)
WTILES = WIN // TILE           # 16 paint tiles per window


def _body(tc, out, feats, seg_full, seg_half, seg_bounce):
    import concourse.mybir as mybir
    from concourse import masks

    dt = mybir.dt
    eq = mybir.AluOpType.is_equal
    mul = mybir.AluOpType.mult
    nc = tc.nc

    with (
        tc.tile_pool(name="const", bufs=1) as cpool,
        tc.tile_pool(name="ft", bufs=4) as ftpool,
        tc.tile_pool(name="oh", bufs=4) as ohpool,
        tc.tile_pool(name="ob", bufs=6) as obpool,
        tc.tile_pool(name="win", bufs=2) as wpool,
        tc.tile_pool(name="pp", bufs=2, space="PSUM") as pp,
        tc.tile_pool(name="po", bufs=4, space="PSUM") as po,
        tc.tile_pool(name="ps", bufs=1, space="PSUM") as ps,
        tc.tile_pool(name="psc", bufs=1, space="PSUM") as psc,
    ):
        # ---- constants ----
        identity = cpool.tile([128, 128], dt.float32)
        masks.make_identity(nc, identity[:])

        iota_row_i = cpool.tile([128, 128], dt.int32)
        nc.gpsimd.iota(iota_row_i[:], pattern=[[1, 128]], base=0, channel_multiplier=0)
        iota_row_f = cpool.tile([128, 128], dt.float32)
        nc.vector.tensor_copy(iota_row_f[:], iota_row_i[:])

        iota_col_i = cpool.tile([128, 1], dt.int32)
        nc.gpsimd.iota(iota_col_i[:], pattern=[[0, 1]], base=0, channel_multiplier=1)
        iota_col_f = cpool.tile([128, 1], dt.float32)
        nc.vector.tensor_copy(iota_col_f[:], iota_col_i[:])

        ones_bf = cpool.tile([1, 128], dt.bfloat16)
        nc.vector.memset(ones_bf[:], 1.0)

        # ---- loads ----
        feats0 = cpool.tile([128, NPIX_DS], dt.float32)
        feats1 = cpool.tile([128, NPIX_DS], dt.float32)
        nc.sync.dma_start(feats0[:], feats[0:128, :])
        nc.sync.dma_start(feats1[:], feats[128:256, :])

        seg_i = cpool.tile([SEG_P, SEG_F], dt.int32)
        nc.sync.dma_start(seg_i[:], seg_half.rearrange("(p f) -> p f", p=SEG_P))
        seg_f = cpool.tile([SEG_P, SEG_F], dt.float32)
        nc.vector.tensor_copy(seg_f[:], seg_i[:])
        seg_bf = cpool.tile([SEG_P, SEG_F], dt.bfloat16)
        nc.vector.tensor_copy(seg_bf[:], seg_f[:])
        # bounce the bf16 ids through DRAM so they can be re-streamed onto
        # partition 0 (matmul operands must start at partition 0/32/64)
        nc.sync.dma_start(seg_bounce.rearrange("(p f) -> p f", p=SEG_P), seg_bf[:])

        # downsampled seg ids, laid out so chunk j / partition p = ds pixel
        # j*128 + p (row-major over the 48x64 grid)
        ds_view = seg_full.rearrange("(h a) (w b) -> a b h w", a=8, b=8)[0, 0]
        ds_i = cpool.tile([128, NCHUNK_DS], dt.int32)
        for p1 in range(2):
            nc.sync.dma_start(
                ds_i[p1 * 64:(p1 + 1) * 64, :],
                ds_view.rearrange("(j p1) c -> p1 c j", p1=2)[p1],
            )
        ds_f = cpool.tile([128, NCHUNK_DS], dt.float32)
        nc.vector.tensor_copy(ds_f[:], ds_i[:])

        # ---- stats: sums + counts via accumulating matmuls ----
        ones_col = cpool.tile([128, 1], dt.float32)
        nc.vector.memset(ones_col[:], 1.0)

        psum_s = ps.tile([128, C], dt.float32)
        psum_c = psc.tile([128, 1], dt.float32)
        for j in range(NCHUNK_DS):
            t0 = pp.tile([128, 128], dt.float32, tag="pp")
            nc.tensor.transpose(t0[:], feats0[:, j * 128:(j + 1) * 128], identity[:])
            t1 = pp.tile([128, 128], dt.float32, tag="pp")
            nc.tensor.transpose(t1[:], feats1[:, j * 128:(j + 1) * 128], identity[:])

            ft = ftpool.tile([128, 256], dt.float32, tag="ft")
            nc.scalar.copy(ft[:, 0:128], t0[:])
            nc.scalar.copy(ft[:, 128:256], t1[:])

            ohd = ohpool.tile([128, 128], dt.float32, tag="oh")
            nc.vector.tensor_scalar(ohd[:], iota_row_f[:], ds_f[:, j:j + 1], None, eq)

            nc.tensor.matmul(
                psum_s[:], ohd[:], ft[:],
                start=(j == 0), stop=(j == NCHUNK_DS - 1),
            )
            nc.tensor.matmul(
                psum_c[:], ohd[:], ones_col[:],
                start=(j == 0), stop=(j == NCHUNK_DS - 1),
            )

        cnt1 = cpool.tile([128, 1], dt.float32)
        nc.vector.tensor_scalar_max(cnt1[:], psum_c[:], 1.0)
        rec = cpool.tile([128, 1], dt.float32)
        nc.vector.reciprocal(rec[:], cnt1[:])
        avg = cpool.tile([128, C], dt.float32r)
        nc.vector.tensor_scalar(avg[:], psum_s[:, 0:256], rec[:], None, mul)

        # ---- paint ----
        win = None
        for t in range(NTILES):
            if t % WTILES == 0:
                g = t // WTILES
                win = wpool.tile([1, WIN], dt.bfloat16, tag="win")
                nc.sync.dma_start(
                    win[:1, :],
                    seg_bounce[g * WIN:(g + 1) * WIN].rearrange("(o f) -> o f", o=1),
                )
            o = (t % WTILES) * TILE
            bc = pp.tile([128, TILE], dt.float32, tag="pp")
            nc.tensor.matmul(
                bc[:], ones_bf[:1, :], win[0:1, o:o + TILE],
                start=True, stop=True,
            )
            oh = ohpool.tile([128, TILE], dt.float32r, tag="oh")
            nc.vector.tensor_scalar(oh[:], bc[:], iota_col_f[:], None, eq)
            for cc in range(2):
                pot = po.tile([128, TILE], dt.float32, tag="po")
                nc.tensor.matmul(
                    pot[:],
                    avg[:, cc * 128:(cc + 1) * 128],
                    oh[:],
                    start=True, stop=True,
                )
                ob = obpool.tile([128, TILE], dt.float32, tag="ob")
                nc.any.tensor_copy(ob[:], pot[:])
                nc.sync.dma_start(
                    out[cc * 128:(cc + 1) * 128, t * TILE:(t + 1) * TILE], ob[:]
                )


def _hoist_extra_matmul_waits(nc):
    """This walrus build allows only one sync-wait per engine instruction.
    Hoist all but one wait onto standalone EventSemaphore instructions
    inserted just before the instruction in the same block."""
    import concourse.mybir as mybir

    n = [0]
    for f in nc.m.functions:
        for b in f.blocks:
            new_insts = []
            changed = False
            for inst in b.instructions:
                si = inst.sync_info
                if (
                    not isinstance(inst, mybir.InstEventSemaphore)
                    and si is not None
                    and si.on_wait is not None
                    and len(si.on_wait) > 1
                ):
                    waits = list(si.on_wait)
                    for w in waits[:-1]:
                        ev = mybir.InstEventSemaphore(
                            name=f"I-hoist-{n[0]}", ins=[], outs=[])
                        n[0] += 1
                        ev.engine = inst.engine
                        ev.sync_info = mybir.SyncInfo(on_wait=[w], on_update=[])
                        new_insts.append(ev)
                    inst.sync_info = mybir.SyncInfo(
                        on_wait=[waits[-1]], on_update=list(si.on_update or []))
                    changed = True
                new_insts.append(inst)
            if changed:
                b.instructions = new_insts


def _build_nc(hoist=True):
    import concourse.bass as bass
    import concourse.mybir as mybir
    import concourse.tile as tile

    dt = mybir.dt
    nc = bass.Bass("TRN2", target_bir_lowering=False, debug=False,
                   enable_asserts=False)
    feats = nc.dram_tensor("feats", [C, NPIX_DS], dt.float32,
                           kind="ExternalInput").ap()
    seg_full = nc.dram_tensor("seg_full", [HIMG, WIMG], dt.int32,
                              kind="ExternalInput").ap()
    seg_half = nc.dram_tensor("seg_half", [NPIX_HALF], dt.int32,
                              kind="ExternalInput").ap()
    out = nc.dram_tensor("out", [C, NPIX_HALF], dt.float32,
                         kind="ExternalOutput").ap()
    seg_bounce = nc.dram_tensor("seg_bounce", [NPIX_HALF], dt.bfloat16).ap()
    with tile.TileContext(nc) as tc:
        _body(tc, out, feats, seg_full, seg_half, seg_bounce)
    if hoist:
        _hoist_extra_matmul_waits(nc)
    return nc


def kernel(F_semantic_features, segmentation_mask, num_total_segments=None):
    global LAST_RESULTS
    from concourse.bass_utils import run_bass_kernel_spmd

    F = np.ascontiguousarray(np.asarray(F_semantic_features, dtype=np.float32))
    seg = np.asarray(segmentation_mask).astype(np.int32)

    if "nc" not in _CACHE:
        _CACHE["nc"] = _build_nc()
    nc = _CACHE["nc"]

    in_maps = []
    for core in range(8):
        b, h = core // 2, core % 2
        in_maps.append({
            "feats": F[b].reshape(C, NPIX_DS),
            "seg_full": np.ascontiguousarray(seg[b]),
            "seg_half": np.ascontiguousarray(
                seg[b, h * HALF_ROWS:(h + 1) * HALF_ROWS, :]).reshape(-1),
        })

    res = run_bass_kernel_spmd(nc, in_maps, core_ids=list(range(8)), trace=TRACE)
    LAST_RESULTS = res

    imgs = []
    for b in range(B):
        top = res.results[2 * b]["out"].reshape(C, HALF_ROWS, WIMG)
        bot = res.results[2 * b + 1]["out"].reshape(C, HALF_ROWS, WIMG)
        imgs.append(np.concatenate([top, bot], axis=1))
    return np.stack(imgs).astype(np.float32)


if __name__ == "__main__":
    rng = np.random.default_rng(0)
    F = rng.standard_normal((B, C, HF, WF), dtype=np.float32)
    seg = rng.integers(0, S, size=(B, HIMG, WIMG)).astype(np.int64)
    outv = kernel(F, seg, S)
    print("out", outv.shape, outv.dtype, float(outv.mean()))


# revision 22
# speedup vs baseline: 444.1975x; 444.1975x over previous
"""Trainium2 Bass kernel: per-image segment-mean repaint (DeepgazeSpade).

Reference computation per image b:
  seg_ds        = segmap[::8, ::8]                  (nearest downsample: 384/48 = 512/64 = 8)
  sums[s, c]    = sum_{p : seg_ds[p] == s} feats[c, p]
  counts[s]     = |{p : seg_ds[p] == s}|
  avg[s, c]     = sums / max(counts, 1)             (0 for empty segments)
  out[c, y, x]  = avg[segmap[y, x], c]

Sharding: 8 cores = 4 images x 2 row-halves (pure data parallel, no
collectives). Each core recomputes the (cheap) per-image segment stats from
the full downsampled grid and paints its own half of the image.

Per-core device algorithm:
  stats: PE-transpose feats [c, pix] -> [pix, c] tiles; build the one-hot
         [pix, s] with tensor_scalar(is_equal) against an iota-row constant;
         24 accumulating matmuls produce [s, 256 sums + counts] in PSUM;
         avg = sums * reciprocal(max(counts, 1)).
  paint: for each 512-pixel tile: a K=1 matmul broadcasts the seg ids to all
         128 partitions (PSUM); tensor_scalar(is_equal) against the
         partition-index iota gives the one-hot [s=128, 512] in SBUF; two
         float32r matmuls with avg as the stationary operand gather all 256
         channels (exact fp32 selection); copy PSUM->SBUF; DMA out.
"""

import numpy as np

B, C = 4, 256
HF, WF = 48, 64
HIMG, WIMG = 384, 512
S = 128
NPIX_DS = HF * WF              # 3072 downsampled pixels
NCHUNK_DS = NPIX_DS // 128     # 24
HALF_ROWS = HIMG // 2          # 192
NPIX_HALF = HALF_ROWS * WIMG   # 98304 pixels per core
SEG_P = 96                     # seg_half SBUF partitions
SEG_F = NPIX_HALF // SEG_P     # 1024
TILE = 512                     # paint tile (one PSUM bank of fp32)
NTILES = NPIX_HALF // TILE     # 192

_CACHE = {}
LAST_RESULTS = None
TRACE = False


WIN = 8192                     # seg-id window on partition 0 (bf16, 16 KiB)
WTILES = WIN // TILE           # 16 paint tiles per window


def _body(tc, out, feats, seg_full, seg_half, seg_bounce):
    import concourse.mybir as mybir
    from concourse import masks

    dt = mybir.dt
    eq = mybir.AluOpType.is_equal
    mul = mybir.AluOpType.mult
    nc = tc.nc

    with (
        tc.tile_pool(name="const", bufs=1) as cpool,
        tc.tile_pool(name="ft", bufs=4) as ftpool,
        tc.tile_pool(name="oh", bufs=4) as ohpool,
        tc.tile_pool(name="ob", bufs=6) as obpool,
        tc.tile_pool(name="win", bufs=2) as wpool,
        tc.tile_pool(name="pp", bufs=2, space="PSUM") as pp,
        tc.tile_pool(name="po", bufs=4, space="PSUM") as po,
        tc.tile_pool(name="ps", bufs=1, space="PSUM") as ps,
        tc.tile_pool(name="psc", bufs=1, space="PSUM") as psc,
    ):
        # ---- constants ----
        identity = cpool.tile([128, 128], dt.float32)
        masks.make_identity(nc, identity[:])

        iota_row_i = cpool.tile([128, 128], dt.int32)
        nc.gpsimd.iota(iota_row_i[:], pattern=[[1, 128]], base=0, channel_multiplier=0)
        iota_row_f = cpool.tile([128, 128], dt.float32)
        nc.vector.tensor_copy(iota_row_f[:], iota_row_i[:])

        iota_col_i = cpool.tile([128, 1], dt.int32)
        nc.gpsimd.iota(iota_col_i[:], pattern=[[0, 1]], base=0, channel_multiplier=1)
        iota_col_f = cpool.tile([128, 1], dt.float32)
        nc.vector.tensor_copy(iota_col_f[:], iota_col_i[:])

        ones_bf = cpool.tile([1, 128], dt.bfloat16)
        nc.vector.memset(ones_bf[:], 1.0)

        # ---- loads ----
        feats0 = cpool.tile([128, NPIX_DS], dt.float32)
        feats1 = cpool.tile([128, NPIX_DS], dt.float32)
        nc.sync.dma_start(feats0[:], feats[0:128, :])
        nc.sync.dma_start(feats1[:], feats[128:256, :])

        seg_i = cpool.tile([SEG_P, SEG_F], dt.int32)
        nc.sync.dma_start(seg_i[:], seg_half.rearrange("(p f) -> p f", p=SEG_P))
        seg_f = cpool.tile([SEG_P, SEG_F], dt.float32)
        nc.vector.tensor_copy(seg_f[:], seg_i[:])
        seg_bf = cpool.tile([SEG_P, SEG_F], dt.bfloat16)
        nc.vector.tensor_copy(seg_bf[:], seg_f[:])
        # bounce the bf16 ids through DRAM so they can be re-streamed onto
        # partition 0 (matmul operands must start at partition 0/32/64)
        nc.sync.dma_start(seg_bounce.rearrange("(p f) -> p f", p=SEG_P), seg_bf[:])

        # downsampled seg ids, laid out so chunk j / partition p = ds pixel
        # j*128 + p (row-major over the 48x64 grid)
        ds_view = seg_full.rearrange("(h a) (w b) -> a b h w", a=8, b=8)[0, 0]
        ds_i = cpool.tile([128, NCHUNK_DS], dt.int32)
        for p1 in range(2):
            nc.sync.dma_start(
                ds_i[p1 * 64:(p1 + 1) * 64, :],
                ds_view.rearrange("(j p1) c -> p1 c j", p1=2)[p1],
            )
        ds_f = cpool.tile([128, NCHUNK_DS], dt.float32)
        nc.vector.tensor_copy(ds_f[:], ds_i[:])

        # ---- stats: sums + counts via accumulating matmuls ----
        ones_col = cpool.tile([128, 1], dt.float32)
        nc.vector.memset(ones_col[:], 1.0)

        psum_s = ps.tile([128, C], dt.float32)
        psum_c = psc.tile([128, 1], dt.float32)
        for j in range(NCHUNK_DS):
            t0 = pp.tile([128, 128], dt.float32, tag="pp")
            nc.tensor.transpose(t0[:], feats0[:, j * 128:(j + 1) * 128], identity[:])
            t1 = pp.tile([128, 128], dt.float32, tag="pp")
            nc.tensor.transpose(t1[:], feats1[:, j * 128:(j + 1) * 128], identity[:])

            ft = ftpool.tile([128, 256], dt.float32, tag="ft")
            nc.scalar.copy(ft[:, 0:128], t0[:])
            nc.scalar.copy(ft[:, 128:256], t1[:])

            ohd = ohpool.tile([128, 128], dt.float32, tag="oh")
            nc.vector.tensor_scalar(ohd[:], iota_row_f[:], ds_f[:, j:j + 1], None, eq)

            nc.tensor.matmul(
                psum_s[:], ohd[:], ft[:],
                start=(j == 0), stop=(j == NCHUNK_DS - 1),
            )
            nc.tensor.matmul(
                psum_c[:], ohd[:], ones_col[:],
                start=(j == 0), stop=(j == NCHUNK_DS - 1),
            )

        cnt1 = cpool.tile([128, 1], dt.float32)
        nc.vector.tensor_scalar_max(cnt1[:], psum_c[:], 1.0)
        rec = cpool.tile([128, 1], dt.float32)
        nc.vector.reciprocal(rec[:], cnt1[:])
        # hi/lo split: float32r matmuls run at full rate but round the
        # stationary operand; two accumulating matmuls (hi + residual)
        # reconstruct full fp32 precision since the one-hot is exact.
        avg_full = cpool.tile([128, C], dt.float32)
        nc.vector.tensor_scalar(avg_full[:], psum_s[:], rec[:], None, mul)
        avg_hi = cpool.tile([128, C], dt.float32r)
        nc.vector.tensor_copy(avg_hi[:], avg_full[:])
        avg_lo = cpool.tile([128, C], dt.float32r)
        nc.vector.tensor_sub(avg_lo[:], avg_full[:], avg_hi[:].bitcast(dt.float32))

        # ---- paint ----
        win = None
        for t in range(NTILES):
            if t % WTILES == 0:
                g = t // WTILES
                win = wpool.tile([1, WIN], dt.bfloat16, tag="win")
                nc.sync.dma_start(
                    win[:1, :],
                    seg_bounce[g * WIN:(g + 1) * WIN].rearrange("(o f) -> o f", o=1),
                )
            o = (t % WTILES) * TILE
            bc = pp.tile([128, TILE], dt.float32, tag="pp")
            nc.tensor.matmul(
                bc[:], ones_bf[:1, :], win[0:1, o:o + TILE],
                start=True, stop=True,
            )
            oh = ohpool.tile([128, TILE], dt.float32r, tag="oh")
            nc.vector.tensor_scalar(oh[:], bc[:], iota_col_f[:], None, eq)
            for cc in range(2):
                sl = slice(cc * 128, (cc + 1) * 128)
                pot = po.tile([128, TILE], dt.float32, tag="po")
                nc.tensor.matmul(
                    pot[:], avg_hi[:, sl], oh[:], start=True, stop=False,
                )
                nc.tensor.matmul(
                    pot[:], avg_lo[:, sl], oh[:], start=False, stop=True,
                )
                ob = obpool.tile([128, TILE], dt.float32, tag="ob")
                nc.any.tensor_copy(ob[:], pot[:])
                nc.sync.dma_start(
                    out[cc * 128:(cc + 1) * 128, t * TILE:(t + 1) * TILE], ob[:]
                )


def _hoist_extra_matmul_waits(nc):
    """This walrus build allows only one sync-wait per engine instruction.
    Hoist all but one wait onto standalone EventSemaphore instructions
    inserted just before the instruction in the same block."""
    import concourse.mybir as mybir

    n = [0]
    for f in nc.m.functions:
        for b in f.blocks:
            new_insts = []
            changed = False
            for inst in b.instructions:
                si = inst.sync_info
                if (
                    not isinstance(inst, mybir.InstEventSemaphore)
                    and si is not None
                    and si.on_wait is not None
                    and len(si.on_wait) > 1
                ):
                    waits = list(si.on_wait)
                    for w in waits[:-1]:
                        ev = mybir.InstEventSemaphore(
                            name=f"I-hoist-{n[0]}", ins=[], outs=[])
                        n[0] += 1
                        ev.engine = inst.engine
                        ev.sync_info = mybir.SyncInfo(on_wait=[w], on_update=[])
                        new_insts.append(ev)
                    inst.sync_info = mybir.SyncInfo(
                        on_wait=[waits[-1]], on_update=list(si.on_update or []))
                    changed = True
                new_insts.append(inst)
            if changed:
                b.instructions = new_insts


def _build_nc(hoist=True, reps=1):
    import concourse.bass as bass
    import concourse.mybir as mybir
    import concourse.tile as tile

    dt = mybir.dt
    nc = bass.Bass("TRN2", target_bir_lowering=False, debug=False,
                   enable_asserts=False)
    feats = nc.dram_tensor("feats", [C, NPIX_DS], dt.float32,
                           kind="ExternalInput").ap()
    seg_full = nc.dram_tensor("seg_full", [HIMG, WIMG], dt.int32,
                              kind="ExternalInput").ap()
    seg_half = nc.dram_tensor("seg_half", [NPIX_HALF], dt.int32,
                              kind="ExternalInput").ap()
    out = nc.dram_tensor("out", [C, NPIX_HALF], dt.float32,
                         kind="ExternalOutput").ap()
    seg_bounce = nc.dram_tensor("seg_bounce", [NPIX_HALF], dt.bfloat16).ap()
    with tile.TileContext(nc) as tc:
        if reps == 1:
            _body(tc, out, feats, seg_full, seg_half, seg_bounce)
        else:
            with tc.For_i(0, reps, 1):
                _body(tc, out, feats, seg_full, seg_half, seg_bounce)
    if hoist:
        _hoist_extra_matmul_waits(nc)
    return nc


def kernel(F_semantic_features, segmentation_mask, num_total_segments=None):
    global LAST_RESULTS
    from concourse.bass_utils import run_bass_kernel_spmd

    F = np.ascontiguousarray(np.asarray(F_semantic_features, dtype=np.float32))
    seg = np.asarray(segmentation_mask).astype(np.int32)

    if "nc" not in _CACHE:
        _CACHE["nc"] = _build_nc()
    nc = _CACHE["nc"]

    in_maps = []
    for core in range(8):
        b, h = core // 2, core % 2
        in_maps.append({
            "feats": F[b].reshape(C, NPIX_DS),
            "seg_full": np.ascontiguousarray(seg[b]),
            "seg_half": np.ascontiguousarray(
                seg[b, h * HALF_ROWS:(h + 1) * HALF_ROWS, :]).reshape(-1),
        })

    res = run_bass_kernel_spmd(nc, in_maps, core_ids=list(range(8)), trace=TRACE)
    LAST_RESULTS = res

    imgs = []
    for b in range(B):
        top = res.results[2 * b]["out"].reshape(C, HALF_ROWS, WIMG)
        bot = res.results[2 * b + 1]["out"].reshape(C, HALF_ROWS, WIMG)
        imgs.append(np.concatenate([top, bot], axis=1))
    return np.stack(imgs).astype(np.float32)


if __name__ == "__main__":
    rng = np.random.default_rng(0)
    F = rng.standard_normal((B, C, HF, WF), dtype=np.float32)
    seg = rng.integers(0, S, size=(B, HIMG, WIMG)).astype(np.int64)
    outv = kernel(F, seg, S)
    print("out", outv.shape, outv.dtype, float(outv.mean()))
